# revision 8
# baseline (speedup 1.0000x reference)
"""Trainium2 Bass kernel for the NeuralODE problem.

Math (matching reference.py):
    20 Euler steps (10 segments x 2 steps, uniform dt => step size hi = 0.05):
        z_{i+1} = z_i + hi * ( tanh(z_i @ W1 + b1 + t_i*wt) @ W2 + b2 )

Device-side reformulation (per core, batch shard B=64):
    - Fold hi into W2:  W2' = hi * W2, c = hi * b2.
    - Keep the "state without accumulated c":  z'_i = z_i - i*c, so
        z'_{i+1} = z'_i + tanh(z'_i @ W1 + bias_i) @ W2'
      with bias_i = b1 + t_i*wt + i*(c @ W1)   (precomputed on host).
      Final output: z_20 = z'_20 + 20*c       (added on host).
    - State kept transposed (d-major) as zT[p, 64k+b] = z'[b, 128k+p] so it can
      be the stationary (lhsT) operand of orientation-B matmuls.
    - Matmuls run in bf16 (1 cyc/col on the PE vs 4 for fp32); the Euler state
      itself stays fp32 in SBUF (bf16 state would absorb the small 0.05*f
      increments), with a bf16 shadow copy produced each step for the PE.
    - Both matmuls stream the (SBUF-resident) weights as the moving operand with
      N=512 chunks; the 64-wide batch stationary only fills half the PE columns,
      so two chunks run concurrently via tile_position col-tiling (0,0)/(0,64).
    - The PE pair rate is LDWEIGHTS-bound at 1 load per matmul (2x131ns >
      512/2.4GHz), so:
        mm1 runs k-major: each z k-tile stationary serves all 4 HID chunks
          (2 concurrent pairs); the redundant 2 LDWEIGHTS are stripped from the
          BIR post-compile (hardware keeps weights loaded across matmuls).
        mm2 alternates k-tiles between the two column positions (even k at
          cols 0-63, odd at 64-127), so each k-tile is loaded once; the two
          per-position partial sums land in different PSUM rows and are folded
          with one DVE add afterwards.
    - The per-step bias enters PSUM first through a K=1 ones-vector matmul.
    - Layout flips (batch-major PSUM result -> d/hid-major stationary for the
      next matmul) are PE transpose-mode matmuls against identity, pipelined in
      128-column blocks: tanh -> transpose -> copy -> dependent matmuls, so the
      serial chains at step boundaries stay short.

Sharding: pure data-parallel over batch (512 -> 8 x 64); weights replicated.
"""

import numpy as np
import ml_dtypes

BS, D, HID = 512, 1024, 2048
NCORES = 8
B = BS // NCORES  # 64
NSTEP = 20
KD = D // 128  # 8 k-tiles for the D contraction
KH = HID // 128  # 16 k-tiles for the HID contraction
F32 = np.float32
BF16 = ml_dtypes.bfloat16

MM_DTYPE = "bfloat16"  # matmul dtype: "bfloat16" (1 cyc/col) or "float32" (4 cyc/col)


def _strip_redundant_ldweights(nc):
    """Remove InstLdweights that reload the stationary already resident at the
    same PE column position (no intervening conflicting load), so consecutive
    matmuls sharing a stationary pay for one LDWEIGHTS. Only sync-free loads
    are stripped; any load carrying waits/updates is kept."""
    import concourse.mybir as mybir

    n_strip = 0
    for func in nc.m.functions:
        for blk in func.blocks:
            loaded = {}  # tile_position -> content key
            keep = []
            for inst in blk.instructions:
                if isinstance(inst, mybir.InstLdweights):
                    ap = inst.ins[0]
                    key = (
                        ap.memref,
                        ap.offset,
                        str(ap.ap),
                        str(getattr(inst, "is_transpose", None)),
                        str(getattr(inst, "tile_size", None)),
                    )
                    tp = tuple(getattr(inst, "tile_position", None) or (0, 0))
                    si = inst.sync_info
                    clean = si is None or (not si.on_wait and not si.on_update)
                    tsz = getattr(inst, "tile_size", None)
                    wide = tsz is None or tsz[1] > 64  # covers both col halves
                    if clean and not wide and loaded.get(tp) == key:
                        n_strip += 1
                        continue
                    if wide:
                        loaded.clear()
                    loaded[tp] = key
                keep.append(inst)
            blk.instructions[:] = keep
    return n_strip


def _build_program(mm_dtype=MM_DTYPE):
    import concourse.mybir as mybir
    from concourse import bacc
    from concourse.tile import TileContext

    nc = bacc.Bacc()
    f32 = mybir.dt.float32
    mmdt = getattr(mybir.dt, mm_dtype)
    lowp = mmdt != f32
    TANH = mybir.ActivationFunctionType.Tanh

    zt_in = nc.dram_tensor("zt_in", [128, KD * B], f32, kind="ExternalInput")
    w1_d = nc.dram_tensor("w1", [128, KD * HID], mmdt, kind="ExternalInput")
    w2_d = nc.dram_tensor("w2", [128, KH * D], mmdt, kind="ExternalInput")
    biases_d = nc.dram_tensor("biases", [NSTEP, HID], mmdt, kind="ExternalInput")
    ident_d = nc.dram_tensor("ident", [128, 128], mmdt, kind="ExternalInput")
    ones_d = nc.dram_tensor("ones", [1, B], mmdt, kind="ExternalInput")
    zt_out = nc.dram_tensor("zt_out", [128, KD * B], f32, kind="ExternalOutput")

    def ublk(t):  # [128, 512] laid out (h, u, c) -> [p, h, u, c]
        return t.rearrange("p (h u c) -> p h u c", h=2, u=4)

    def tblk(t):  # transpose-psum [128, 512] laid out (u, h, c) -> [p, h, u, c]
        return t.rearrange("p (u h c) -> p h u c", u=4, h=2)

    with (
        TileContext(nc) as tc,
        tc.tile_pool(name="const", bufs=1) as cpool,
        tc.tile_pool(name="weights", bufs=1) as wpool,
        tc.tile_pool(name="state", bufs=1) as spool,
        tc.tile_pool(name="work", bufs=2) as hpool,
        tc.tile_pool(name="bias", bufs=2) as bpool,
        tc.tile_pool(name="psumh", bufs=1, space="PSUM") as ph_pool,
        tc.tile_pool(name="psumt", bufs=1, space="PSUM") as pt_pool,
        tc.tile_pool(name="psumf", bufs=1, space="PSUM") as pf_pool,
    ):
        ident_sb = cpool.tile([128, 128], mmdt, tag="ident")
        nc.sync.dma_start(ident_sb[:], ident_d[:])
        ones_sb = cpool.tile([1, B], mmdt, tag="ones")
        nc.sync.dma_start(ones_sb[:], ones_d[:])

        zt = spool.tile([128, KD * B], f32, tag="zt")  # fp32 z'_T state [128, 512]
        nc.sync.dma_start(zt[:], zt_in[:])
        if lowp:
            zmm = spool.tile([128, KD * B], mmdt, tag="zmm")  # bf16 shadow for PE
            nc.vector.tensor_copy(zmm[:], zt[:])
        else:
            zmm = zt
        hT = spool.tile([128, KH * B], mmdt, tag="hT")  # tanh'd h, hid-major [128,1024]

        # per-k weight tiles so step-0 matmuls can start as soon as their
        # own k-slice has landed (whole-tensor deps would stall ~25us)
        w1t = []
        for k in range(KD):
            w = wpool.tile([128, HID], mmdt, tag=f"w1_{k}")
            nc.sync.dma_start(w[:], w1_d[:, k * HID : (k + 1) * HID])
            w1t.append(w)
        w2t = []
        for k in range(KH):
            w = wpool.tile([128, D], mmdt, tag=f"w2_{k}")
            nc.sync.dma_start(w[:], w2_d[:, k * D : (k + 1) * D])
            w2t.append(w)

        # mm1 k-order: u-major so each k needs only the state u-block that the
        # boundary pipeline has produced most recently (k = 4h + u).
        K_ORDER = [0, 4, 1, 5, 2, 6, 3, 7]

        def mm1_bias(phs, bias_sb):
            for c in range(4):
                nc.tensor.matmul(
                    phs[c // 2][64 * (c % 2) : 64 * (c % 2) + 64, :],
                    ones_sb[:1, :],
                    bias_sb[:1, 512 * c : 512 * c + 512],
                    start=True,
                    stop=False,
                    tile_position=(0, 64 * (c % 2)),
                )

        def mm1_kgroup(phs, ki):
            # one z k-tile stationary (both col positions) serves all 4 HID
            # chunks; the redundant LDWEIGHTS are stripped post-compile.
            k = K_ORDER[ki]
            last = ki == KD - 1
            for c in range(4):
                nc.tensor.matmul(
                    phs[c // 2][64 * (c % 2) : 64 * (c % 2) + 64, :],
                    zmm[:, B * k : B * k + B],
                    w1t[k][:, 512 * c : 512 * c + 512],
                    start=False,
                    stop=last,
                    tile_position=(0, 64 * (c % 2)),
                )

        def mid_block(phs, h_bms, pts, g, u):
            # tanh -> PE transpose -> hT copy for one 128-col block
            sl = slice(128 * u, 128 * u + 128)
            nc.scalar.activation(h_bms[g][:, sl], phs[g][:, sl], TANH)
            nc.tensor.matmul(
                pts[g][:, sl],
                h_bms[g][:, sl],
                ident_sb[:],
                is_transpose=True,
                start=True,
                stop=True,
            )
            nc.vector.tensor_copy(
                ublk(hT[:, 512 * g : 512 * g + 512])[:, :, u : u + 1, :],
                tblk(pts[g][:])[:, :, u : u + 1, :],
            )

        def mm2_kpair(pfs, kA, kB, first, last):
            # kA even-u block at cols 0-63, kB odd-u block at cols 64-127; the
            # two per-position partial sums fold after the loop.
            for c in range(2):
                nc.tensor.matmul(
                    pfs[c][0:64, 512 * 0 : 512],
                    hT[:, B * kA : B * kA + B],
                    w2t[kA][:, 512 * c : 512 * c + 512],
                    start=first,
                    stop=last,
                    tile_position=(0, 0),
                )
                nc.tensor.matmul(
                    pfs[c][64:128, :],
                    hT[:, B * kB : B * kB + B],
                    w2t[kB][:, 512 * c : 512 * c + 512],
                    start=first,
                    stop=last,
                    tile_position=(0, 64),
                )

        def fold_block(pfs, f_bm, f_tmp, u):
            # one PSUM read per instruction: copy the odd-position partial out
            # first, then add it to the even-position partial.
            sl = slice(128 * u, 128 * u + 128)
            for c in range(2):
                nc.vector.tensor_copy(
                    f_tmp[64 * c : 64 * c + 64, sl], pfs[c][64:128, sl]
                )
            for c in range(2):
                nc.vector.tensor_add(
                    f_bm[64 * c : 64 * c + 64, sl],
                    pfs[c][0:64, sl],
                    f_tmp[64 * c : 64 * c + 64, sl],
                )

        def tf_block(f_bm, pt2, u):
            sl = slice(128 * u, 128 * u + 128)
            nc.tensor.matmul(
                pt2[:, sl],
                f_bm[:, sl],
                ident_sb[:],
                is_transpose=True,
                start=True,
                stop=True,
            )

        def zmm_block(pt2, u):
            nc.vector.tensor_add(
                ublk(zmm[:])[:, :, u : u + 1, :],
                ublk(zt[:])[:, :, u : u + 1, :],
                tblk(pt2[:])[:, :, u : u + 1, :],
            )

        prev = None  # (pfs, f_bm, f_tmp, pt2) of the previous step
        for i in range(NSTEP):
            bias_sb = bpool.tile([1, HID], mmdt, tag="bias")
            nc.sync.dma_start(bias_sb[:], biases_d[i : i + 1, :])

            ph_a = ph_pool.tile([128, 512], f32, tag="phA")
            ph_b = ph_pool.tile([128, 512], f32, tag="phB")
            phs = [ph_a, ph_b]

            # ---- boundary of the previous step interleaved with this mm1 ----
            # Emission order staggers producers (DVE folds, PE transposes, DVE
            # state adds) one block ahead of the consuming mm1 k-groups so no
            # PE instruction waits at the head of the queue.
            mm1_bias(phs, bias_sb)
            if prev is None:
                for ki in range(KD):
                    mm1_kgroup(phs, ki)
            elif not lowp:
                p_pfs, p_fbm, p_ftmp, p_pt2 = prev
                for u in range(4):
                    fold_block(p_pfs, p_fbm, p_ftmp, u)
                    tf_block(p_fbm, p_pt2, u)
                nc.vector.tensor_add(ublk(zt[:]), ublk(zt[:]), tblk(p_pt2[:]))
                for ki in range(KD):
                    mm1_kgroup(phs, ki)
            else:
                p_pfs, p_fbm, p_ftmp, p_pt2 = prev
                fold_block(p_pfs, p_fbm, p_ftmp, 0)
                tf_block(p_fbm, p_pt2, 0)
                fold_block(p_pfs, p_fbm, p_ftmp, 1)
                tf_block(p_fbm, p_pt2, 1)
                zmm_block(p_pt2, 0)
                mm1_kgroup(phs, 0)  # k0 (u0)
                fold_block(p_pfs, p_fbm, p_ftmp, 2)
                tf_block(p_fbm, p_pt2, 2)
                zmm_block(p_pt2, 1)
                mm1_kgroup(phs, 1)  # k4 (u0)
                fold_block(p_pfs, p_fbm, p_ftmp, 3)
                tf_block(p_fbm, p_pt2, 3)
                zmm_block(p_pt2, 2)
                mm1_kgroup(phs, 2)  # k1 (u1)
                zmm_block(p_pt2, 3)
                for ki in range(3, KD):
                    mm1_kgroup(phs, ki)
                # fp32 master state update; off the critical path (next mm1
                # only needs zmm). After the zmm reads of zt (DVE is FIFO).
                nc.vector.tensor_add(ublk(zt[:]), ublk(zt[:]), tblk(p_pt2[:]))

            # ---- tanh/transpose/copy blocks feeding mm2, with one-block lag ----
            h_bm0 = hpool.tile([128, 512], mmdt, tag="h_bm0")
            h_bm1 = hpool.tile([128, 512], mmdt, tag="h_bm1")
            h_bms = [h_bm0, h_bm1]
            pt_0 = pt_pool.tile([128, 512], mmdt, tag="pt0")
            pt_1 = pt_pool.tile([128, 512], mmdt, tag="pt1")
            pts = [pt_0, pt_1]
            pf_a = pf_pool.tile([128, 512], f32, tag="pfA")
            pf_b = pf_pool.tile([128, 512], f32, tag="pfB")
            pfs = [pf_a, pf_b]

            mid_block(phs, h_bms, pts, 0, 0)
            mid_block(phs, h_bms, pts, 0, 1)
            mid_block(phs, h_bms, pts, 0, 2)
            mm2_kpair(pfs, 0, 1, True, False)  # needs (g0,u0),(g0,u1)
            mm2_kpair(pfs, 4, 5, False, False)
            mid_block(phs, h_bms, pts, 0, 3)
            mid_block(phs, h_bms, pts, 1, 0)
            mm2_kpair(pfs, 2, 3, False, False)  # needs (g0,u2),(g0,u3)
            mm2_kpair(pfs, 6, 7, False, False)
            mid_block(phs, h_bms, pts, 1, 1)
            mid_block(phs, h_bms, pts, 1, 2)
            mm2_kpair(pfs, 8, 9, False, False)  # needs (g1,u0),(g1,u1)
            mm2_kpair(pfs, 12, 13, False, False)
            mid_block(phs, h_bms, pts, 1, 3)
            mm2_kpair(pfs, 10, 11, False, False)  # needs (g1,u2),(g1,u3)
            mm2_kpair(pfs, 14, 15, False, True)

            f_bm = hpool.tile([128, 512], mmdt, tag="f_bm")
            f_tmp = hpool.tile([128, 512], mmdt, tag="f_tmp")
            pt2 = pt_pool.tile([128, 512], mmdt, tag="pt2")
            prev = (pfs, f_bm, f_tmp, pt2)

        # final boundary: fold/transpose/update for the last step
        p_pfs, p_fbm, p_ftmp, p_pt2 = prev
        for u in range(4):
            fold_block(p_pfs, p_fbm, p_ftmp, u)
            tf_block(p_fbm, p_pt2, u)
        nc.vector.tensor_add(ublk(zt[:]), ublk(zt[:]), tblk(p_pt2[:]))

        nc.sync.dma_start(zt_out[:], zt[:])

    nc.compile()
    stripped = _strip_redundant_ldweights(nc)
    assert stripped >= NSTEP * 2 * (KD - 1), f"only stripped {stripped} ldweights"
    return nc


def _pack_zT(shard):  # [B, D] -> [128, KD*B]
    return np.ascontiguousarray(
        shard.T.reshape(KD, 128, B).transpose(1, 0, 2).reshape(128, KD * B)
    )


def _unpack_zT(zt):  # [128, KD*B] -> [B, D]
    return zt.reshape(128, KD, B).transpose(1, 0, 2).reshape(D, B).T


def _host_inputs(z0, t, W1, b1, wt, W2, b2, npdt):
    t = np.asarray(t, F32)
    t0s, t1s = t[:-1], t[1:]
    h_seg = (t1s - t0s) / 2.0  # N_STEPS_PER_SEG = 2
    step_ts = (t0s[:, None] + h_seg[:, None] * np.arange(2, dtype=F32)[None, :]).reshape(
        -1
    )
    step_hs = np.repeat(h_seg, 2)
    assert np.allclose(step_hs, step_hs[0]), "non-uniform Euler steps unsupported"
    scale = F32(step_hs[0])

    c = (scale * np.asarray(b2, F32)).astype(F32)  # [D]
    cW1 = (c.astype(np.float64) @ np.asarray(W1, np.float64)).astype(F32)  # [HID]
    biases = np.stack(
        [
            (np.asarray(b1, F32) + step_ts[i] * np.asarray(wt, F32) + i * cW1).astype(
                F32
            )
            for i in range(NSTEP)
        ]
    ).astype(npdt)  # [NSTEP, HID]

    w1p = np.ascontiguousarray(
        np.asarray(W1, F32).reshape(KD, 128, HID).transpose(1, 0, 2).reshape(128, KD * HID)
    ).astype(npdt)
    w2p = np.ascontiguousarray(
        (scale * np.asarray(W2, F32))
        .astype(F32)
        .reshape(KH, 128, D)
        .transpose(1, 0, 2)
        .reshape(128, KH * D)
    ).astype(npdt)
    ident = np.eye(128, dtype=npdt)
    ones = np.ones((1, B), npdt)
    return biases, w1p, w2p, ident, ones, c


def _make_in_maps(z0, t, W1, b1, wt, W2, b2, npdt):
    z0 = np.asarray(z0, F32)
    biases, w1p, w2p, ident, ones, c = _host_inputs(z0, t, W1, b1, wt, W2, b2, npdt)
    in_maps = []
    for core in range(NCORES):
        shard = z0[core * B : (core + 1) * B]
        in_maps.append(
            {
                "zt_in": _pack_zT(shard),
                "w1": w1p,
                "w2": w2p,
                "biases": biases,
                "ident": ident,
                "ones": ones,
            }
        )
    return in_maps, c


def run(z0, t, W1, b1, wt, W2, b2, trace=False, mm_dtype=MM_DTYPE):
    from concourse.bass_utils import run_bass_kernel_spmd

    npdt = F32 if mm_dtype == "float32" else BF16
    in_maps, c = _make_in_maps(z0, t, W1, b1, wt, W2, b2, npdt)
    nc = _build_program(mm_dtype=mm_dtype)
    res = run_bass_kernel_spmd(nc, in_maps, core_ids=list(range(NCORES)), trace=trace)

    outs = []
    for core in range(NCORES):
        z_shard = _unpack_zT(np.asarray(res.results[core]["zt_out"], F32))
        outs.append(z_shard)
    out = np.concatenate(outs, axis=0).astype(F32)
    out = out + (NSTEP * c)[None, :].astype(F32)
    return out.astype(F32), res


def kernel(z0, t, W1, b1, wt, W2, b2):
    out, _ = run(z0, t, W1, b1, wt, W2, b2, trace=False)
    return out


# revision 11
# speedup vs baseline: 1.4440x; 1.4440x over previous
"""Trainium2 Bass kernel for the NeuralODE problem.

Math (matching reference.py):
    20 Euler steps (10 segments x 2 steps, uniform dt => step size hi = 0.05):
        z_{i+1} = z_i + hi * ( tanh(z_i @ W1 + b1 + t_i*wt) @ W2 + b2 )

Device-side reformulation (per core, batch shard B=64):
    - Fold hi into W2:  W2' = hi * W2, c = hi * b2.
    - Keep the "state without accumulated c":  z'_i = z_i - i*c, so
        z'_{i+1} = z'_i + tanh(z'_i @ W1 + bias_i) @ W2'
      with bias_i = b1 + t_i*wt + i*(c @ W1)   (precomputed on host).
      Final output: z_20 = z'_20 + 20*c       (added on host).
    - State kept transposed (d-major) as zT[p, 64k+b] = z'[b, 128k+p] so it can
      be the stationary (lhsT) operand of orientation-B matmuls.
    - Matmuls run in bf16 (1 cyc/col on the PE vs 4 for fp32); the Euler state
      itself stays fp32 in SBUF (bf16 state would absorb the small 0.05*f
      increments), with a bf16 shadow copy produced each step for the PE.
    - Both matmuls stream the (SBUF-resident) weights as the moving operand with
      N=512 chunks; the 64-wide batch stationary only fills half the PE columns,
      so two chunks run concurrently via tile_position col-tiling (0,0)/(0,64).
    - The PE pair rate is LDWEIGHTS-bound at 1 load per matmul (2x131ns >
      512/2.4GHz), so:
        mm1 runs k-major: each z k-tile stationary serves all 4 HID chunks
          (2 concurrent pairs); the redundant 2 LDWEIGHTS are stripped from the
          BIR post-compile (hardware keeps weights loaded across matmuls).
        mm2 alternates k-tiles between the two column positions (even k at
          cols 0-63, odd at 64-127), so each k-tile is loaded once; the two
          per-position partial sums land in different PSUM rows and are folded
          with one DVE add afterwards.
    - The per-step bias enters PSUM first through a K=1 ones-vector matmul.
    - Layout flips (batch-major PSUM result -> d/hid-major stationary for the
      next matmul) are PE transpose-mode matmuls against identity, pipelined in
      128-column blocks: tanh -> transpose -> copy -> dependent matmuls, so the
      serial chains at step boundaries stay short.

Sharding: pure data-parallel over batch (512 -> 8 x 64); weights replicated.
"""

import numpy as np
import ml_dtypes

BS, D, HID = 512, 1024, 2048
NCORES = 8
B = BS // NCORES  # 64
NSTEP = 20
KD = D // 128  # 8 k-tiles for the D contraction
KH = HID // 128  # 16 k-tiles for the HID contraction
F32 = np.float32
BF16 = ml_dtypes.bfloat16

MM_DTYPE = "bfloat16"  # matmul dtype: "bfloat16" (1 cyc/col) or "float32" (4 cyc/col)


def _strip_redundant_ldweights(nc):
    """Remove InstLdweights that reload the stationary already resident at the
    same PE column position (no intervening conflicting load), so consecutive
    matmuls sharing a stationary pay for one LDWEIGHTS. Only sync-free loads
    are stripped; any load carrying waits/updates is kept."""
    import concourse.mybir as mybir

    n_strip = 0
    for func in nc.m.functions:
        for blk in func.blocks:
            loaded = {}  # tile_position -> content key
            keep = []
            for inst in blk.instructions:
                if isinstance(inst, mybir.InstLdweights):
                    ap = inst.ins[0]
                    key = (
                        ap.memref,
                        ap.offset,
                        str(ap.ap),
                        str(getattr(inst, "is_transpose", None)),
                        str(getattr(inst, "tile_size", None)),
                    )
                    tp = tuple(getattr(inst, "tile_position", None) or (0, 0))
                    si = inst.sync_info
                    clean = si is None or (not si.on_wait and not si.on_update)
                    tsz = getattr(inst, "tile_size", None)
                    wide = tsz is None or tsz[1] > 64  # covers both col halves
                    if clean and not wide and loaded.get(tp) == key:
                        n_strip += 1
                        continue
                    if wide:
                        loaded.clear()
                    loaded[tp] = key
                keep.append(inst)
            blk.instructions[:] = keep
    return n_strip


def _build_program(mm_dtype=MM_DTYPE):
    import concourse.mybir as mybir
    from concourse import bacc
    from concourse.tile import TileContext

    nc = bacc.Bacc()
    f32 = mybir.dt.float32
    mmdt = getattr(mybir.dt, mm_dtype)
    lowp = mmdt != f32
    TANH = mybir.ActivationFunctionType.Tanh

    zt_in = nc.dram_tensor("zt_in", [128, KD * B], f32, kind="ExternalInput")
    w1_d = nc.dram_tensor("w1", [128, KD * HID], mmdt, kind="ExternalInput")
    w2_d = nc.dram_tensor("w2", [128, KH * D], mmdt, kind="ExternalInput")
    biases_d = nc.dram_tensor("biases", [NSTEP, HID], mmdt, kind="ExternalInput")
    ident_d = nc.dram_tensor("ident", [128, 128], mmdt, kind="ExternalInput")
    ones_d = nc.dram_tensor("ones", [1, B], mmdt, kind="ExternalInput")
    zt_out = nc.dram_tensor("zt_out", [128, KD * B], f32, kind="ExternalOutput")

    def ublk(t):  # [128, 512] laid out (h, u, c) -> [p, h, u, c]
        return t.rearrange("p (h u c) -> p h u c", h=2, u=4)

    def tblk(t):  # transpose-psum [128, 512] laid out (u, h, c) -> [p, h, u, c]
        return t.rearrange("p (u h c) -> p h u c", u=4, h=2)

    with (
        TileContext(nc) as tc,
        tc.tile_pool(name="const", bufs=1) as cpool,
        tc.tile_pool(name="weights", bufs=1) as wpool,
        tc.tile_pool(name="state", bufs=1) as spool,
        tc.tile_pool(name="work", bufs=2) as hpool,
        tc.tile_pool(name="bias", bufs=2) as bpool,
        tc.tile_pool(name="psumh", bufs=1, space="PSUM") as ph_pool,
        tc.tile_pool(name="psumt", bufs=1, space="PSUM") as pt_pool,
        tc.tile_pool(name="psumf", bufs=1, space="PSUM") as pf_pool,
    ):
        ident_sb = cpool.tile([128, 128], mmdt, tag="ident")
        nc.sync.dma_start(ident_sb[:], ident_d[:])
        ones_sb = cpool.tile([1, B], mmdt, tag="ones")
        nc.sync.dma_start(ones_sb[:], ones_d[:])

        zt = spool.tile([128, KD * B], f32, tag="zt")  # fp32 z'_T state [128, 512]
        nc.sync.dma_start(zt[:], zt_in[:])
        if lowp:
            zmm = spool.tile([128, KD * B], mmdt, tag="zmm")  # bf16 shadow for PE
            nc.vector.tensor_copy(zmm[:], zt[:])
        else:
            zmm = zt
        hT = spool.tile([128, KH * B], mmdt, tag="hT")  # tanh'd h, hid-major [128,1024]

        # per-k weight tiles so step-0 matmuls can start as soon as their
        # own k-slice has landed (whole-tensor deps would stall ~25us)
        w1t = []
        for k in range(KD):
            w = wpool.tile([128, HID], mmdt, tag=f"w1_{k}")
            nc.sync.dma_start(w[:], w1_d[:, k * HID : (k + 1) * HID])
            w1t.append(w)
        w2t = []
        for k in range(KH):
            w = wpool.tile([128, D], mmdt, tag=f"w2_{k}")
            nc.sync.dma_start(w[:], w2_d[:, k * D : (k + 1) * D])
            w2t.append(w)

        # mm1 k-order: A-half k's (u-blocks 0,1 -> k in {0,1,4,5}) first, so the
        # first half of mm1 only needs the state half the boundary pipeline
        # produces first (k = 4h + u).
        K_ORDER = [0, 1, 4, 5, 2, 3, 6, 7]

        def mm1_bias(phs, bias_sb):
            for c in range(4):
                nc.tensor.matmul(
                    phs[c // 2][64 * (c % 2) : 64 * (c % 2) + 64, :],
                    ones_sb[:1, :],
                    bias_sb[:1, 512 * c : 512 * c + 512],
                    start=True,
                    stop=False,
                    tile_position=(0, 64 * (c % 2)),
                )

        def mm1_kgroup(phs, ki):
            # one z k-tile stationary (both col positions) serves all 4 HID
            # chunks; the redundant LDWEIGHTS are stripped post-compile.
            k = K_ORDER[ki]
            last = ki == KD - 1
            for c in range(4):
                nc.tensor.matmul(
                    phs[c // 2][64 * (c % 2) : 64 * (c % 2) + 64, :],
                    zmm[:, B * k : B * k + B],
                    w1t[k][:, 512 * c : 512 * c + 512],
                    start=False,
                    stop=last,
                    tile_position=(0, 64 * (c % 2)),
                )

        def mm2_k(pf, k, first, last):
            # chunk 0 (d cols 0-511) in psum rows 0-63 via position (0,0),
            # chunk 1 in rows 64-127 via (0,64); one hT k-tile stationary at
            # both positions.
            nc.tensor.matmul(
                pf[0:64, :],
                hT[:, B * k : B * k + B],
                w2t[k][:, 0:512],
                start=first,
                stop=last,
                tile_position=(0, 0),
            )
            nc.tensor.matmul(
                pf[64:128, :],
                hT[:, B * k : B * k + B],
                w2t[k][:, 512:1024],
                start=first,
                stop=last,
                tile_position=(0, 64),
            )

        def copy_f(pf, f_bm, half):
            # f (batch-major, chunk-row-packed) PSUM -> SBUF bf16, one u-pair
            sl = slice(256 * half, 256 * half + 256)
            nc.vector.tensor_copy(f_bm[:, sl], pf[:, sl])

        def tf_block(f_bm, pt2, u):
            sl = slice(128 * u, 128 * u + 128)
            nc.tensor.matmul(
                pt2[:, sl],
                f_bm[:, sl],
                ident_sb[:],
                is_transpose=True,
                start=True,
                stop=True,
            )

        def zmm_half(pt2, half):
            # state shadow for u-blocks {2*half, 2*half+1} = k in {0,1,4,5} or
            # {2,3,6,7}
            us = slice(2 * half, 2 * half + 2)
            nc.vector.tensor_add(
                ublk(zmm[:])[:, :, us, :],
                ublk(zt[:])[:, :, us, :],
                tblk(pt2[:])[:, :, us, :],
            )

        def hT_copy(pts, g, half):
            us = slice(2 * half, 2 * half + 2)
            nc.vector.tensor_copy(
                ublk(hT[:, 512 * g : 512 * g + 512])[:, :, us, :],
                tblk(pts[g][:])[:, :, us, :],
            )

        prev = None  # (pf, f_bm, pt2) of the previous step
        for i in range(NSTEP):
            bias_sb = bpool.tile([1, HID], mmdt, tag="bias")
            nc.sync.dma_start(bias_sb[:], biases_d[i : i + 1, :])

            ph_a = ph_pool.tile([128, 512], f32, tag="phA")
            ph_b = ph_pool.tile([128, 512], f32, tag="phB")
            phs = [ph_a, ph_b]

            # ---- previous step's state update interleaved with this mm1 ----
            # The f transposes and state-shadow adds are split in halves so the
            # first half of mm1's k-groups start after ~1us of boundary chain;
            # emission staggers producers ahead of consumers.
            mm1_bias(phs, bias_sb)
            if prev is None:
                for ki in range(KD):
                    mm1_kgroup(phs, ki)
            else:
                p_pf, p_fbm, p_pt2 = prev
                copy_f(p_pf, p_fbm, 0)
                tf_block(p_fbm, p_pt2, 0)
                tf_block(p_fbm, p_pt2, 1)
                if lowp:
                    zmm_half(p_pt2, 0)
                copy_f(p_pf, p_fbm, 1)
                mm1_kgroup(phs, 0)  # k0
                tf_block(p_fbm, p_pt2, 2)
                mm1_kgroup(phs, 1)  # k1
                tf_block(p_fbm, p_pt2, 3)
                if lowp:
                    zmm_half(p_pt2, 1)
                mm1_kgroup(phs, 2)  # k4
                mm1_kgroup(phs, 3)  # k5
                # fp32 master state update; off the critical path (next mm1
                # only needs zmm). After the zmm reads of zt (DVE is FIFO).
                nc.vector.tensor_add(ublk(zt[:]), ublk(zt[:]), tblk(p_pt2[:]))
                for ki in range(4, KD):
                    mm1_kgroup(phs, ki)  # k2, k3, k6, k7

            # ---- tanh -> transpose -> hT copy feeding mm2 ----
            h_bm0 = hpool.tile([128, 512], mmdt, tag="h_bm0")
            h_bm1 = hpool.tile([128, 512], mmdt, tag="h_bm1")
            h_bms = [h_bm0, h_bm1]
            pt_0 = pt_pool.tile([128, 512], mmdt, tag="pt0")
            pt_1 = pt_pool.tile([128, 512], mmdt, tag="pt1")
            pts = [pt_0, pt_1]
            pf = pf_pool.tile([128, 512], f32, tag="pf")

            MM2_ORDER = [0, 1, 4, 5, 2, 3, 6, 7, 8, 9, 12, 13, 10, 11, 14, 15]
            nc.scalar.activation(h_bms[0][:], phs[0][:], TANH)
            for u in range(4):
                nc.tensor.matmul(
                    pts[0][:, 128 * u : 128 * u + 128],
                    h_bms[0][:, 128 * u : 128 * u + 128],
                    ident_sb[:],
                    is_transpose=True,
                    start=True,
                    stop=True,
                )
            hT_copy(pts, 0, 0)
            hT_copy(pts, 0, 1)
            nc.scalar.activation(h_bms[1][:], phs[1][:], TANH)
            for ki in range(8):  # g0 k-tiles; tanh g1 overlaps on ACT
                mm2_k(pf, MM2_ORDER[ki], ki == 0, False)
            for u in range(4):
                nc.tensor.matmul(
                    pts[1][:, 128 * u : 128 * u + 128],
                    h_bms[1][:, 128 * u : 128 * u + 128],
                    ident_sb[:],
                    is_transpose=True,
                    start=True,
                    stop=True,
                )
            hT_copy(pts, 1, 0)
            hT_copy(pts, 1, 1)
            for ki in range(8, 16):
                mm2_k(pf, MM2_ORDER[ki], False, ki == 15)

            f_bm = hpool.tile([128, 512], mmdt, tag="f_bm")
            pt2 = pt_pool.tile([128, 512], mmdt, tag="pt2")
            prev = (pf, f_bm, pt2)

        # final boundary: fold/transpose/update for the last step
        p_pf, p_fbm, p_pt2 = prev
        copy_f(p_pf, p_fbm, 0)
        copy_f(p_pf, p_fbm, 1)
        for u in range(4):
            tf_block(p_fbm, p_pt2, u)
        nc.vector.tensor_add(ublk(zt[:]), ublk(zt[:]), tblk(p_pt2[:]))

        nc.sync.dma_start(zt_out[:], zt[:])

    nc.compile()
    stripped = _strip_redundant_ldweights(nc)
    assert stripped >= NSTEP * 2 * (KD - 1), f"only stripped {stripped} ldweights"
    return nc


def _pack_zT(shard):  # [B, D] -> [128, KD*B]
    return np.ascontiguousarray(
        shard.T.reshape(KD, 128, B).transpose(1, 0, 2).reshape(128, KD * B)
    )


def _unpack_zT(zt):  # [128, KD*B] -> [B, D]
    return zt.reshape(128, KD, B).transpose(1, 0, 2).reshape(D, B).T


def _host_inputs(z0, t, W1, b1, wt, W2, b2, npdt):
    t = np.asarray(t, F32)
    t0s, t1s = t[:-1], t[1:]
    h_seg = (t1s - t0s) / 2.0  # N_STEPS_PER_SEG = 2
    step_ts = (t0s[:, None] + h_seg[:, None] * np.arange(2, dtype=F32)[None, :]).reshape(
        -1
    )
    step_hs = np.repeat(h_seg, 2)
    assert np.allclose(step_hs, step_hs[0]), "non-uniform Euler steps unsupported"
    scale = F32(step_hs[0])

    c = (scale * np.asarray(b2, F32)).astype(F32)  # [D]
    cW1 = (c.astype(np.float64) @ np.asarray(W1, np.float64)).astype(F32)  # [HID]
    biases = np.stack(
        [
            (np.asarray(b1, F32) + step_ts[i] * np.asarray(wt, F32) + i * cW1).astype(
                F32
            )
            for i in range(NSTEP)
        ]
    ).astype(npdt)  # [NSTEP, HID]

    w1p = np.ascontiguousarray(
        np.asarray(W1, F32).reshape(KD, 128, HID).transpose(1, 0, 2).reshape(128, KD * HID)
    ).astype(npdt)
    w2p = np.ascontiguousarray(
        (scale * np.asarray(W2, F32))
        .astype(F32)
        .reshape(KH, 128, D)
        .transpose(1, 0, 2)
        .reshape(128, KH * D)
    ).astype(npdt)
    ident = np.eye(128, dtype=npdt)
    ones = np.ones((1, B), npdt)
    return biases, w1p, w2p, ident, ones, c


def _make_in_maps(z0, t, W1, b1, wt, W2, b2, npdt):
    z0 = np.asarray(z0, F32)
    biases, w1p, w2p, ident, ones, c = _host_inputs(z0, t, W1, b1, wt, W2, b2, npdt)
    in_maps = []
    for core in range(NCORES):
        shard = z0[core * B : (core + 1) * B]
        in_maps.append(
            {
                "zt_in": _pack_zT(shard),
                "w1": w1p,
                "w2": w2p,
                "biases": biases,
                "ident": ident,
                "ones": ones,
            }
        )
    return in_maps, c


def run(z0, t, W1, b1, wt, W2, b2, trace=False, mm_dtype=MM_DTYPE):
    from concourse.bass_utils import run_bass_kernel_spmd

    npdt = F32 if mm_dtype == "float32" else BF16
    in_maps, c = _make_in_maps(z0, t, W1, b1, wt, W2, b2, npdt)
    nc = _build_program(mm_dtype=mm_dtype)
    res = run_bass_kernel_spmd(nc, in_maps, core_ids=list(range(NCORES)), trace=trace)

    outs = []
    for core in range(NCORES):
        z_shard = _unpack_zT(np.asarray(res.results[core]["zt_out"], F32))
        outs.append(z_shard)
    out = np.concatenate(outs, axis=0).astype(F32)
    out = out + (NSTEP * c)[None, :].astype(F32)
    return out.astype(F32), res


def kernel(z0, t, W1, b1, wt, W2, b2):
    out, _ = run(z0, t, W1, b1, wt, W2, b2, trace=False)
    return out


# revision 17
# speedup vs baseline: 1.7167x; 1.1889x over previous
"""Trainium2 Bass kernel for the NeuralODE problem.

Math (matching reference.py):
    20 Euler steps (10 segments x 2 steps, uniform dt => step size hi = 0.05):
        z_{i+1} = z_i + hi * ( tanh(z_i @ W1 + b1 + t_i*wt) @ W2 + b2 )

Device-side reformulation (per core, batch shard B=64):
    - Fold hi into W2:  W2' = hi * W2, c = hi * b2, and absorb the accumulated
      c into the output:  z'_i = z_i - i*c  evolves with bias_i = b1 + i*v,
      v = hi*wt + (c @ W1);  final output z_20 = z'_20 + 20*c (added on host).
    - The whole recursion runs in h-space: with h_pre_i = z'_i @ W1 + bias_i
      and M = W2' @ W1 (precomputed on host),
          h_pre_{i+1} = h_pre_i + tanh(h_pre_i) @ M + v
          z'_20 = z'_0 + (sum_i tanh(h_pre_i)) @ W2'
      so the sequential core is a single 2048x2048 matmul per step; z' and the
      f-transposes leave the loop entirely.
    - h_pre lives in two PSUM banks in fp32 for all 20 steps; the per-step
      matmul and the constant-v ones-matmul accumulate onto it in place
      (start=False), so there is no state copy at all.  S (the tanh sum) is
      accumulated batch-major in fp32 by the DVE, off the critical path.
    - Matmuls run in bf16 (1 cyc/col); the moving operand is the SBUF-resident
      M in N=512 chunks; the 64-wide batch stationary (tanh'd h, transposed)
      fills the two PE column halves via tile_position (0,0)/(0,64), and each
      stationary load serves all 4 output chunks (the redundant LDWEIGHTS the
      bass legalizer emits per-matmul are stripped from the BIR post-compile).
    - Layout flips (batch-major tanh output -> hid-major stationary) are PE
      transpose-mode matmuls against identity, batched per 512-col group and
      interleaved with the consuming matmuls.

Sharding: pure data-parallel over batch (512 -> 8 x 64); weights replicated.
"""

import numpy as np
import ml_dtypes

BS, D, HID = 512, 1024, 2048
NCORES = 8
B = BS // NCORES  # 64
NSTEP = 20
KD = D // 128  # 8 k-tiles for the D contraction
KH = HID // 128  # 16 k-tiles for the HID contraction
F32 = np.float32
BF16 = ml_dtypes.bfloat16

MM_DTYPE = "bfloat16"


def _strip_redundant_ldweights(nc):
    """Remove InstLdweights that reload the stationary already resident at the
    same PE column position (no intervening conflicting load), so consecutive
    matmuls sharing a stationary pay for one LDWEIGHTS. Only sync-free loads
    are stripped; any load carrying waits/updates is kept."""
    import concourse.mybir as mybir

    n_strip = 0
    for func in nc.m.functions:
        for blk in func.blocks:
            loaded = {}  # tile_position -> content key
            keep = []
            for inst in blk.instructions:
                if isinstance(inst, mybir.InstLdweights):
                    ap = inst.ins[0]
                    key = (
                        ap.memref,
                        ap.offset,
                        str(ap.ap),
                        str(getattr(inst, "is_transpose", None)),
                        str(getattr(inst, "tile_size", None)),
                    )
                    tp = tuple(getattr(inst, "tile_position", None) or (0, 0))
                    si = inst.sync_info
                    clean = si is None or (not si.on_wait and not si.on_update)
                    tsz = getattr(inst, "tile_size", None)
                    wide = tsz is None or tsz[1] > 64  # covers both col halves
                    if clean and not wide and loaded.get(tp) == key:
                        n_strip += 1
                        continue
                    if wide:
                        loaded.clear()
                    loaded[tp] = key
                keep.append(inst)
            blk.instructions[:] = keep
    return n_strip


def _build_program(mm_dtype=MM_DTYPE):
    import concourse.mybir as mybir
    from concourse import bacc
    from concourse.tile import TileContext

    assert mm_dtype == "bfloat16"
    nc = bacc.Bacc()
    f32 = mybir.dt.float32
    mmdt = mybir.dt.bfloat16
    TANH = mybir.ActivationFunctionType.Tanh

    zt_in = nc.dram_tensor("zt_in", [128, KD * B], f32, kind="ExternalInput")
    w1_d = nc.dram_tensor("w1", [128, KD * HID], mmdt, kind="ExternalInput")
    m_d = nc.dram_tensor("m", [128, KH * HID], mmdt, kind="ExternalInput")
    w2_d = nc.dram_tensor("w2", [128, KH * D], mmdt, kind="ExternalInput")
    b0_d = nc.dram_tensor("bias0", [1, HID], mmdt, kind="ExternalInput")
    v_d = nc.dram_tensor("vvec", [1, HID], mmdt, kind="ExternalInput")
    ident_d = nc.dram_tensor("ident", [128, 128], mmdt, kind="ExternalInput")
    ones_d = nc.dram_tensor("ones", [1, B], mmdt, kind="ExternalInput")
    zt_out = nc.dram_tensor("zt_out", [128, KD * B], f32, kind="ExternalOutput")

    def ublk(t):  # [128, 512] laid out (h, u, c) -> [p, h, u, c]
        return t.rearrange("p (h u c) -> p h u c", h=2, u=4)

    def tblk(t):  # transpose-psum [128, 512] laid out (u, h, c) -> [p, h, u, c]
        return t.rearrange("p (u h c) -> p h u c", u=4, h=2)

    with (
        TileContext(nc) as tc,
        tc.tile_pool(name="const", bufs=1) as cpool,
        tc.tile_pool(name="weights", bufs=1) as wpool,
        tc.tile_pool(name="state", bufs=1) as spool,
        tc.tile_pool(name="work", bufs=2) as hpool,
        tc.tile_pool(name="psumh", bufs=1, space="PSUM") as ph_pool,
        tc.tile_pool(name="psumt", bufs=1, space="PSUM") as pt_pool,
        tc.tile_pool(name="psumf", bufs=1, space="PSUM") as pf_pool,
    ):
        ident_sb = cpool.tile([128, 128], mmdt, tag="ident")
        nc.sync.dma_start(ident_sb[:], ident_d[:])
        ones_sb = cpool.tile([1, B], mmdt, tag="ones")
        nc.sync.dma_start(ones_sb[:], ones_d[:])
        b0_sb = cpool.tile([1, HID], mmdt, tag="b0")
        nc.sync.dma_start(b0_sb[:], b0_d[:])
        v_sb = cpool.tile([1, HID], mmdt, tag="v")
        nc.sync.dma_start(v_sb[:], v_d[:])

        zt = spool.tile([128, KD * B], f32, tag="zt")  # fp32 z'_0, updated once
        nc.sync.dma_start(zt[:], zt_in[:])
        zmm = spool.tile([128, KD * B], mmdt, tag="zmm")  # bf16 z'_0 for step 0
        nc.vector.tensor_copy(zmm[:], zt[:])
        hT = spool.tile([128, KH * B], mmdt, tag="hT")  # h/S, hid-major
        S0 = spool.tile([128, 512], f32, tag="S0")  # sum of tanh, batch-major
        S1 = spool.tile([128, 512], f32, tag="S1")
        Ss = [S0, S1]

        w1t = []
        for k in range(KD):
            w = wpool.tile([128, HID], mmdt, tag=f"w1_{k}")
            nc.sync.dma_start(w[:], w1_d[:, k * HID : (k + 1) * HID])
            w1t.append(w)
        mt = []
        for k in range(KH):
            w = wpool.tile([128, HID], mmdt, tag=f"m_{k}")
            nc.sync.dma_start(w[:], m_d[:, k * HID : (k + 1) * HID])
            mt.append(w)
        w2t = []
        for k in range(KH):
            w = wpool.tile([128, D], mmdt, tag=f"w2_{k}")
            nc.sync.dma_start(w[:], w2_d[:, k * D : (k + 1) * D])
            w2t.append(w)

        # h_pre state: two PSUM banks, fp32, resident for the whole scan.
        # chunk c (512 HID cols) -> bank c//2, rows 64*(c%2).
        ph_a = ph_pool.tile([128, 512], f32, tag="phA")
        ph_b = ph_pool.tile([128, 512], f32, tag="phB")
        phs = [ph_a, ph_b]

        def bias_seed(bias_sb, start):
            for c in range(4):
                nc.tensor.matmul(
                    phs[c // 2][64 * (c % 2) : 64 * (c % 2) + 64, :],
                    ones_sb[:1, :],
                    bias_sb[:1, 512 * c : 512 * c + 512],
                    start=start,
                    stop=False,
                    skip_group_check=True,
                    tile_position=(0, 64 * (c % 2)),
                )

        def acc_kgroup(stat, statk, wt, k):
            # one stationary k-tile (both col positions) serves all 4 chunks;
            # redundant LDWEIGHTS stripped post-compile.  Accumulates onto the
            # resident h_pre psum state.
            for c in range(4):
                nc.tensor.matmul(
                    phs[c // 2][64 * (c % 2) : 64 * (c % 2) + 64, :],
                    stat[:, B * statk : B * statk + B],
                    wt[k][:, 512 * c : 512 * c + 512],
                    start=False,
                    stop=False,
                    skip_group_check=True,
                    tile_position=(0, 64 * (c % 2)),
                )

        # ---- step 0: h_pre_0 = z'_0 @ W1 + bias_0 ----
        bias_seed(b0_sb, True)
        for k in range(KD):
            acc_kgroup(zmm, k, w1t, k)

        for i in range(NSTEP):
            h_bm0 = hpool.tile([128, 512], mmdt, tag="h_bm0")
            h_bm1 = hpool.tile([128, 512], mmdt, tag="h_bm1")
            h_bms = [h_bm0, h_bm1]
            last = i == NSTEP - 1

            nc.scalar.activation(h_bms[0][:], phs[0][:], TANH)
            if not last:
                pt_0 = pt_pool.tile([128, 512], mmdt, tag="pt0")
                for u in range(4):
                    nc.tensor.matmul(
                        pt_0[:, 128 * u : 128 * u + 128],
                        h_bms[0][:, 128 * u : 128 * u + 128],
                        ident_sb[:],
                        is_transpose=True,
                        start=True,
                        stop=True,
                    )
                nc.vector.tensor_copy(
                    ublk(hT[:, 0:512]), tblk(pt_0[:])
                )
            nc.scalar.activation(h_bms[1][:], phs[1][:], TANH)
            if i == 0:
                nc.vector.tensor_copy(Ss[0][:], h_bms[0][:])
            else:
                nc.vector.tensor_add(Ss[0][:], Ss[0][:], h_bms[0][:])
            if not last:
                pt_1 = pt_pool.tile([128, 512], mmdt, tag="pt1")
                for u in range(4):
                    nc.tensor.matmul(
                        pt_1[:, 128 * u : 128 * u + 128],
                        h_bms[1][:, 128 * u : 128 * u + 128],
                        ident_sb[:],
                        is_transpose=True,
                        start=True,
                        stop=True,
                    )
                nc.vector.tensor_copy(
                    ublk(hT[:, 512:1024]), tblk(pt_1[:])
                )
            if i == 0:
                nc.vector.tensor_copy(Ss[1][:], h_bms[1][:])
            else:
                nc.vector.tensor_add(Ss[1][:], Ss[1][:], h_bms[1][:])

            if not last:
                # ---- h_pre += tanh(h_pre) @ M + v, in place in PSUM ----
                bias_seed(v_sb, False)
                for k in range(KH):  # g0 tiles first; T g1 runs during k0-7
                    acc_kgroup(hT, k, mt, k)

        # ---- z'_20 = z'_0 + S @ W2' ----
        sb0 = hpool.tile([128, 512], mmdt, tag="sb0")
        nc.vector.tensor_copy(sb0[:], Ss[0][:])
        sb1 = hpool.tile([128, 512], mmdt, tag="sb1")
        nc.vector.tensor_copy(sb1[:], Ss[1][:])
        sbs = [sb0, sb1]
        for g in range(2):
            pt_s = pt_pool.tile([128, 512], mmdt, tag="pts")
            for u in range(4):
                nc.tensor.matmul(
                    pt_s[:, 128 * u : 128 * u + 128],
                    sbs[g][:, 128 * u : 128 * u + 128],
                    ident_sb[:],
                    is_transpose=True,
                    start=True,
                    stop=True,
                )
            nc.vector.tensor_copy(
                ublk(hT[:, 512 * g : 512 * g + 512]), tblk(pt_s[:])
            )
        pf = pf_pool.tile([128, 512], f32, tag="pf")
        for k in range(KH):
            nc.tensor.matmul(
                pf[0:64, :],
                hT[:, B * k : B * k + B],
                w2t[k][:, 0:512],
                start=(k == 0),
                stop=(k == KH - 1),
                tile_position=(0, 0),
            )
            nc.tensor.matmul(
                pf[64:128, :],
                hT[:, B * k : B * k + B],
                w2t[k][:, 512:1024],
                start=(k == 0),
                stop=(k == KH - 1),
                tile_position=(0, 64),
            )
        f_bm = hpool.tile([128, 512], mmdt, tag="f_bm")
        nc.vector.tensor_copy(f_bm[:], pf[:])
        pt2 = pt_pool.tile([128, 512], mmdt, tag="pt2")
        for u in range(4):
            nc.tensor.matmul(
                pt2[:, 128 * u : 128 * u + 128],
                f_bm[:, 128 * u : 128 * u + 128],
                ident_sb[:],
                is_transpose=True,
                start=True,
                stop=True,
            )
        nc.vector.tensor_add(ublk(zt[:]), ublk(zt[:]), tblk(pt2[:]))

        nc.sync.dma_start(zt_out[:], zt[:])

    nc.compile()
    stripped = _strip_redundant_ldweights(nc)
    assert stripped >= (NSTEP - 1) * 2 * (KH - 1) // 2, (
        f"only stripped {stripped} ldweights"
    )
    return nc


def _pack_zT(shard):  # [B, D] -> [128, KD*B]
    return np.ascontiguousarray(
        shard.T.reshape(KD, 128, B).transpose(1, 0, 2).reshape(128, KD * B)
    )


def _unpack_zT(zt):  # [128, KD*B] -> [B, D]
    return zt.reshape(128, KD, B).transpose(1, 0, 2).reshape(D, B).T


def _host_inputs(z0, t, W1, b1, wt, W2, b2, npdt):
    t = np.asarray(t, F32)
    t0s, t1s = t[:-1], t[1:]
    h_seg = (t1s - t0s) / 2.0  # N_STEPS_PER_SEG = 2
    step_ts = (t0s[:, None] + h_seg[:, None] * np.arange(2, dtype=F32)[None, :]).reshape(
        -1
    )
    step_hs = np.repeat(h_seg, 2)
    assert np.allclose(step_hs, step_hs[0]), "non-uniform Euler steps unsupported"
    assert np.allclose(step_ts, step_ts[0] + step_hs[0] * np.arange(NSTEP)), (
        "non-linear step time grid unsupported"
    )
    scale = F32(step_hs[0])

    W1_64 = np.asarray(W1, np.float64)
    W2_64 = np.asarray(W2, np.float64)
    c = (scale * np.asarray(b2, F32)).astype(F32)  # [D]
    cW1 = (c.astype(np.float64) @ W1_64).astype(F32)  # [HID]
    bias0 = (
        np.asarray(b1, F32) + step_ts[0] * np.asarray(wt, F32)
    ).astype(F32)[None, :]
    vvec = (scale * np.asarray(wt, F32) + cW1).astype(F32)[None, :]

    M = (float(scale) * (W2_64 @ W1_64)).astype(F32)  # [HID, HID]
    m_p = np.ascontiguousarray(
        M.reshape(KH, 128, HID).transpose(1, 0, 2).reshape(128, KH * HID)
    ).astype(npdt)
    w1p = np.ascontiguousarray(
        np.asarray(W1, F32).reshape(KD, 128, HID).transpose(1, 0, 2).reshape(128, KD * HID)
    ).astype(npdt)
    w2p = np.ascontiguousarray(
        (scale * np.asarray(W2, F32))
        .astype(F32)
        .reshape(KH, 128, D)
        .transpose(1, 0, 2)
        .reshape(128, KH * D)
    ).astype(npdt)
    ident = np.eye(128, dtype=npdt)
    ones = np.ones((1, B), npdt)
    return bias0.astype(npdt), vvec.astype(npdt), m_p, w1p, w2p, ident, ones, c


def _make_in_maps(z0, t, W1, b1, wt, W2, b2, npdt):
    z0 = np.asarray(z0, F32)
    bias0, vvec, m_p, w1p, w2p, ident, ones, c = _host_inputs(
        z0, t, W1, b1, wt, W2, b2, npdt
    )
    in_maps = []
    for core in range(NCORES):
        shard = z0[core * B : (core + 1) * B]
        in_maps.append(
            {
                "zt_in": _pack_zT(shard),
                "w1": w1p,
                "m": m_p,
                "w2": w2p,
                "bias0": bias0,
                "vvec": vvec,
                "ident": ident,
                "ones": ones,
            }
        )
    return in_maps, c


def run(z0, t, W1, b1, wt, W2, b2, trace=False, mm_dtype=MM_DTYPE):
    from concourse.bass_utils import run_bass_kernel_spmd

    npdt = BF16
    in_maps, c = _make_in_maps(z0, t, W1, b1, wt, W2, b2, npdt)
    nc = _build_program(mm_dtype=mm_dtype)
    res = run_bass_kernel_spmd(nc, in_maps, core_ids=list(range(NCORES)), trace=trace)

    outs = []
    for core in range(NCORES):
        z_shard = _unpack_zT(np.asarray(res.results[core]["zt_out"], F32))
        outs.append(z_shard)
    out = np.concatenate(outs, axis=0).astype(F32)
    out = out + (NSTEP * c)[None, :].astype(F32)
    return out.astype(F32), res


def kernel(z0, t, W1, b1, wt, W2, b2):
    out, _ = run(z0, t, W1, b1, wt, W2, b2, trace=False)
    return out
